# revision 51
# baseline (speedup 1.0000x reference)
import sys
for _p in ("/opt/trn_rl_repo",):
    if _p not in sys.path:
        sys.path.insert(0, _p)
"""Bass/Tile multi-head attention kernel for TRN2, head-sharded across 8 cores.

Math (per core c, heads h0=2c, h1=2c+1, dims slice dd = [128c : 128c+128]):
  QT = (Wq[dd] @ q^T + bq[dd])          # [128, R]  (dims on partitions)
  KT likewise; V^T-direct: per k-slice, stationary = vT activation d-tile,
  moving = Wv^T tile -> [128 k-rows, 128 dims] lands straight in vaug layout
  (both heads side by side, ones col per head for the Z row) - no transposes.
  scoresT = K_h Q_h^T / 8               # [k, q] tiles, dk=64 contraction
  attnT = exp(scoresT)                  # no max-subtraction (scores ~ N(0,1))
  outT_aug = V_aug^T @ attnT            # [65, q]; row 64 = Z (softmax denom)
  concatT[h*64:(h+1)*64] = outT_aug[:64] / Z   # stacked both heads [128, q]
  out_projT = Wo[:, dd]^T-part @ concatT       # [1024, R] partial, host sums

Host: feeds qT/kT/vT (pre-transposed [D, R], f16), per-core weights fused
into blob tensors split by first-use time (Wq first, Wk behind the q0 load,
Wv early-late, Wo late); sums the 8 f16 partial out_projT results, adds bo,
transposes back. f16 everywhere: fp8 on any Q/K/V/attn path was measured at
8-10% output error (weight-relative errors do not average down) vs the 2e-2
budget; f16 keeps max rel err at 1.2e-3.

Scheduling: one flat sc/exp emission cursor runs `look` attnV-steps ahead
ACROSS r-block boundaries (at-tiles buffered in the atp pool), so the final
block's exps run during the previous block's attnV phase and ACT never
becomes the end-of-kernel pacer; at most one emission follows each
interleaved prep/lazy closure so exp-paced sc groups never sit back-to-back
in the PE FIFO. Normalize/out-proj/store closures go through a lazy global
work queue drained at 2 closures per attention ki-step. 1/Z rides a
single-partition DVE reciprocal + Pool partition_broadcast in steady state
(no PE), and a PE ones-matmul broadcast in the tail (PE idle there, lower
latency). Timing notes: per-matmul weight-load overhead on HW punishes many
small matmuls; a V-path via separate PE transposes or XBAR DMA transposes
both measured slower on HW than V^T-direct.
"""
import numpy as np
from collections import deque

import concourse.bass as bass
import concourse.bacc as bacc
import concourse.mybir as mybir
import concourse.tile as tile

F32 = mybir.dt.float32
F32R = mybir.dt.float32r
F16 = mybir.dt.float16
EXP = mybir.ActivationFunctionType.Exp
CPY = mybir.ActivationFunctionType.Copy

P = 128
DK = 64
D = 1024
DO = D // P          # 8 contraction tiles for projections
NCORES = 8
RBLK = 512           # r-block (free-dim streaming chunk)

# fused const blob column layout (f16, [128, *] / [65, *])
W_WQ = 0
W_WK = W_WQ + DO * P
W_ID = W_WK + DO * P
W_COLS = W_ID + P
M_BQ = 0             # misc blob, partition 0 rows unless noted
M_BK = M_BQ + P
M_ONES = M_BK + P
M_ONESZ = M_ONES + RBLK      # partitions 0..64
M_COLS = M_ONESZ + DK
L_WV = 0
L_WO = L_WV + DO * P
L_BV = L_WO + DO * P         # partition 0
L_COLS = L_BV + P


def build_kernel(B=2, S=2048, reps=1, in_dt="f16", lazy=True,
                 split_last=None, with_bias=False, RSV=0, look=8,
                 qh0=2, kh0=2, ps_swap=False, ATPB=18, PERT=0, IOB=5, MIDB=3):
    """Returns nc. Same program for all 8 cores (SPMD); per-core data
    differs only in the weight slices fed by the host."""
    R = B * S
    NRB = R // RBLK          # r-blocks total
    RBPB = S // RBLK         # r-blocks per batch
    NKT = S // P             # k-slices (128 rows) per batch
    NCT = D // P             # output column tiles
    HW = RBLK // 2           # half-width for the final block's two passes

    DTI = F16 if in_dt == "f16" else F32R
    DTM = F16                # intermediate dtype (SBUF tensors fed to PE)
    nc = bacc.Bacc("TRN2", target_bir_lowering=False, debug=False,
                   num_devices=NCORES)

    def din(name, shape, dt=DTM):
        return nc.dram_tensor(name, shape, dt, kind="ExternalInput").ap()

    qT = din("qT", [D, R], DTI)
    kT = din("kT", [D, R], DTI)
    vT = din("vT", [D, R], DTI)
    blob_w = din("blob_w", [P, W_COLS], DTI)
    blob_m = din("blob_m", [DK + 1, M_COLS], DTI)
    blob_l = din("blob_l", [P, L_COLS], DTM)
    outT = nc.dram_tensor("outT", [D, R], F16, kind="ExternalOutput").ap()

    qT_r = qT.rearrange("(do p) r -> p do r", p=P)
    kT_r = kT.rearrange("(do p) r -> p do r", p=P)
    vT_r = vT.rearrange("(do p) r -> p do r", p=P)
    outT_r = outT.rearrange("(ct p) r -> p ct r", p=P)

    with tile.TileContext(nc) as tc:
        with tc.tile_pool(name="const", bufs=1) as const, \
             tc.tile_pool(name="io", bufs=IOB) as io, \
             tc.tile_pool(name="mid", bufs=MIDB) as mid, \
             tc.tile_pool(name="ccp", bufs=4) as ccp, \
             tc.tile_pool(name="atp", bufs=ATPB) as atp, \
             tc.tile_pool(name="opp", bufs=4) as opp, \
             tc.tile_pool(name="ps_main", bufs=2, space="PSUM") as ps_main, \
             tc.tile_pool(name="ps_sc", bufs=1 if ps_swap else 2,
                          space="PSUM") as ps_sc, \
             tc.tile_pool(name="ps_out", bufs=2 if ps_swap else 1,
                          space="PSUM") as ps_out:

            # --- constants: three fused blob DMAs (early weights, small
            # misc rows, late weights) instead of ~10 separate transfers ---
            wblob = const.tile([P, W_COLS], DTI, tag="wblob")
            mblob = const.tile([DK + 1, M_COLS], DTI, tag="mblob")
            lblob = const.tile([P, L_COLS], DTM, tag="lblob")
            warm_sb = const.tile([P, P], DTM, tag="warm")
            KT_sb = const.tile([P, R], DTM, tag="KT")
            # per-batch V^T store: [k-part, kt, head, dk+1]; slice [:, ki, h, :]
            # is the [128, 65] attnV lhsT (ones in col DK for the Z row)
            vaugb = [const.tile([P, NKT, 2, DK + 1], DTM, tag=f"vaug{b}",
                                name=f"vaug{b}") for b in range(B)]

            wq_sb = wblob[:, W_WQ:W_WQ + DO * P].rearrange("p (do d) -> p do d", do=DO)
            wk_sb = wblob[:, W_WK:W_WK + DO * P].rearrange("p (do d) -> p do d", do=DO)
            bq_sb = mblob[0:1, M_BQ:M_BQ + P]
            bk_sb = mblob[0:1, M_BK:M_BK + P]
            ones_sb = mblob[0:1, M_ONES:M_ONES + RBLK]
            onesZ_sb = mblob[:, M_ONESZ:M_ONESZ + DK]
            wv_sb = lblob[:, L_WV:L_WV + DO * P].rearrange("p (do d) -> p do d", do=DO)
            wo_sb = lblob[:, L_WO:L_WO + DO * P].rearrange("p (ct c) -> p ct c", ct=NCT)
            bv_sb = lblob[0:1, L_BV:L_BV + P]

            # DVE memset: no Pool q7-launch latency ahead of the PE warmup
            nc.vector.memset(warm_sb[:], 0.0)
            # Wq first so the q0-projection's operands land earliest; Wk can
            # arrive during the q-projection
            nc.sync.dma_start(wblob[:, W_WQ:W_WQ + DO * P],
                              blob_w[:, W_WQ:W_WQ + DO * P])
            if with_bias:
                nc.sync.dma_start(mblob[:], blob_m)

            # Warm the PE (HAM clock gate / p-state ramp) on a memset tile —
            # no DMA dependency, so it runs while the first loads stream.
            def pe_warmup():
                wps = ps_main.tile([P, P], F32, tag="proj", name="warm")
                for i in range(24):
                    nc.tensor.matmul(wps[:], warm_sb[:], warm_sb[:],
                                     start=True, stop=True)

            def const_late():
                # Wv (+bv) feeds the v0 projection soon; Wo isn't read until
                # the first normalize (~25us) — keep it out of this DMA
                nc.sync.dma_start(lblob[:, L_WV:L_WV + DO * P],
                                  blob_l[:, L_WV:L_WV + DO * P])
                if with_bias:
                    nc.sync.dma_start(lblob[:, L_BV:L_BV + P],
                                      blob_l[:, L_BV:L_BV + P])
                if not with_bias:
                    # only onesZ is needed from the misc blob — off the
                    # startup critical DMA chain
                    nc.sync.dma_start(mblob[:], blob_m)
                for b in range(B):
                    nc.gpsimd.memset(vaugb[b][:, :, :, DK:DK + 1], 1.0)

            def const_wo():
                nc.sync.dma_start(lblob[:, L_WO:L_WO + DO * P],
                                  blob_l[:, L_WO:L_WO + DO * P])

            # out[p_out, free] = lhsT.T @ rhs: lhsT = weight tile (output dims
            # on its free axis), rhs = transposed-activation block (rows on
            # free axis), contraction over D on partitions.
            def proj2(src_r, w_sb, b_sb, rb, halves=1):
                """Project one r-block. halves=2 issues two half-width DMA +
                matmul groups so the first output columns are ready one
                half-transfer earlier (used for the very first q-block)."""
                ps = ps_main.tile([P, RBLK], F32, tag="proj")
                hw2 = RBLK // halves
                for i in range(halves):
                    t = io.tile([P, DO, hw2], DTI, tag="io")
                    nc.sync.dma_start(
                        t[:], src_r[:, :, rb * RBLK + i * hw2:
                                     rb * RBLK + (i + 1) * hw2])
                    cols = slice(i * hw2, (i + 1) * hw2)
                    for do in range(DO):
                        nc.tensor.matmul(ps[:, cols], w_sb[:, do], t[:, do],
                                         start=(do == 0),
                                         stop=(not with_bias and do == DO - 1))
                    if with_bias:
                        nc.tensor.matmul(ps[:, cols], b_sb, ones_sb[:, cols],
                                         start=False, stop=True)
                return ps

            # ---- Stage A: K/V projections; V transposed via XBAR DMA ----
            # 512-wide moving projections amortize the per-matmul weight load
            # the PE pays on HW; the [dims, rows] -> [rows, dims] reshape for
            # vaug rides the DMA crossbar (~8 xbar tiles per k-slice), off
            # the PE entirely.
            # ---- Stage A: K projection + V^T-direct projection ----
            # V is projected in transposed orientation: stationary = the vT
            # activation d-tile [128 d, 128 rows], moving = Wv^T tile
            # [128 d, 128 dims] -> out [128 rows, 128 dims] lands directly in
            # vaug layout (both heads side by side), no PE transposes.
            def stage_a_parts(rb):
                b = rb // RBPB
                parts = []

                def kpart():
                    ps_k = proj2(kT_r, wk_sb, bk_sb, rb,
                                 halves=kh0 if rb == 0 else 1)
                    nc.vector.tensor_copy(
                        KT_sb[:, rb * RBLK:(rb + 1) * RBLK], ps_k[:])
                parts.append(kpart)

                v_box = []

                def vload():
                    t = io.tile([P, DO, RBLK], DTI, tag="io", name="tv")
                    nc.sync.dma_start(
                        t[:], vT_r[:, :, rb * RBLK:(rb + 1) * RBLK])
                    ps_v = ps_main.tile([P, RBLK // P, P], F32, tag="proj",
                                        name="psv")
                    v_box.extend((t, ps_v))
                parts.append(vload)

                def vpart(rc):
                    def f():
                        t, ps_v = v_box
                        kt_i = (rb % RBPB) * (RBLK // P) + rc
                        for do in range(DO):
                            nc.tensor.matmul(
                                ps_v[:, rc, :],
                                t[:, do, rc * P:(rc + 1) * P],
                                wv_sb[:, do],
                                start=(do == 0),
                                stop=(not with_bias and do == DO - 1))
                        if with_bias:
                            nc.tensor.matmul(
                                ps_v[:, rc, :], ones_sb[:, 0:P], bv_sb,
                                start=False, stop=True)
                        nc.vector.tensor_copy(
                            vaugb[b][:, kt_i, :, 0:DK],
                            ps_v[:, rc, :].rearrange("p (h d) -> p h d", h=2))
                    return f
                for rc in range(RBLK // P):
                    parts.append(vpart(rc))
                return parts

            # ---- Stage B: Q proj + attention ----
            qtbs = {}
            pos = {}
            work_q = deque()     # lazy normalize/out-proj/store closures

            def stage_q_part(rb, halves=1):
                def f():
                    ps_q = proj2(qT_r, wq_sb, bq_sb, rb, halves=halves)
                    qtb = mid.tile([P, RBLK], DTM, tag="qtb", name="qtb")
                    nc.vector.tensor_copy(qtb[:], ps_q[:])
                    qtbs[rb] = qtb
                return [f]

            def emit_scexp(rb, ki, off, w):
                # paired heads: one [128, 2, w] scores tile, one exp
                b = rb // RBPB
                qtb = qtbs[rb]
                scp = ps_sc.tile([P, 2, RBLK], F32, tag="sc", name="sc")
                for h in (0, 1):
                    hs = slice(h * DK, (h + 1) * DK)
                    reps_mm = 2 if PERT == 2 else 1
                    for _ in range(reps_mm):
                        nc.tensor.matmul(
                            scp[:, h, 0:w],
                            KT_sb[hs, b * S + ki * P: b * S + (ki + 1) * P],
                            qtb[hs, off:off + w], start=True, stop=True)
                at = atp.tile([P, 2, RBLK], DTM, tag="at", name="at")
                nc.scalar.activation(at[:, :, 0:w], scp[:, :, 0:w],
                                     EXP, scale=0.125)
                if PERT == 1:
                    at2 = atp.tile([P, 2, RBLK], DTM, tag="at", name="at2")
                    nc.scalar.activation(at2[:, :, 0:w], scp[:, :, 0:w],
                                         EXP, scale=0.25)
                return at

            def stage_n_parts(key, rb, off=0, w=RBLK, tail=False,
                              po_off=0, keep_po=False):
                """Normalize + out-project cols [off, off+w) of block rb.
                Returns (pre_parts, lazy_parts). `tail` spreads the work
                across engines (ACT/Pool idle there, DVE is the chain)."""
                pre = []
                parts = []
                osbs = []

                def ncopy(h):
                    def f():
                        po = pos[key][h]
                        osb = mid.tile([DK + 1, RBLK], DTM, tag=f"osb{h}",
                                       name=f"osb{h}")
                        nc.vector.tensor_copy(osb[:, 0:w],
                                              po[:, po_off:po_off + w])
                        osbs.append(osb)
                    return f

                cc_box = []

                def npart(h):
                    def f():
                        if not cc_box:
                            cc_box.append(ccp.tile([P, RBLK], DTM, tag="cc",
                                                   name="cc"))
                        osb = osbs[h]
                        zr = mid.tile([DK, RBLK], DTM, tag=f"zr{h}", name=f"zr{h}")
                        if tail:
                            # tail: PE is idle and chain latency is exposed —
                            # broadcast Z via a PE matmul (107ns) instead of
                            # recip->Pool-broadcast (~1.2us)
                            zb = ps_main.tile([DK, RBLK], F32, tag="proj",
                                              name="zb")
                            nc.tensor.matmul(zb[:, 0:w], onesZ_sb[DK:DK + 1, :],
                                             osb[DK:DK + 1, 0:w],
                                             start=True, stop=True)
                            with nc.allow_low_precision(reason="1/Z in f16: rel 5e-4, budget 2e-2"):
                                nc.vector.reciprocal(zr[:, 0:w], zb[:, 0:w])
                        else:
                            # steady state: 1/Z on the single Z row, then Pool
                            # broadcasts across the 64 head partitions — keeps
                            # the PE out of the normalize chain entirely
                            zr1 = mid.tile([1, RBLK], DTM, tag=f"zr1{h}",
                                           name=f"zr1{h}")
                            with nc.allow_low_precision(reason="1/Z in f16: rel 5e-4, budget 2e-2"):
                                nc.vector.reciprocal(zr1[:, 0:w],
                                                     osb[DK:DK + 1, 0:w])
                            nc.gpsimd.partition_broadcast(zr[:, 0:w],
                                                          zr1[:, 0:w])
                        # stacked concat: head h lands on partitions
                        # [64h, 64h+64) so the out-proj contracts both heads
                        # in a single 128-deep matmul per column tile.
                        nc.vector.tensor_mul(
                            cc_box[0][h * DK:(h + 1) * DK, 0:w],
                            osb[0:DK, 0:w], zr[:, 0:w])
                    return f
                pre.append(ncopy(0))
                pre.append(ncopy(1))
                parts.append(npart(0))
                parts.append(npart(1))

                op_box = []

                def alloc_op():
                    op_box.append(opp.tile([P, NCT, RBLK], F16, tag="op_sb",
                                           name="op_sb"))
                parts.append(alloc_op)

                def oppart(ct):
                    def f():
                        op = ps_main.tile([P, RBLK], F32, tag="proj", name="op")
                        nc.tensor.matmul(op[:, 0:w], wo_sb[:, ct],
                                         cc_box[0][:, 0:w],
                                         start=True, stop=True)
                        if tail and ct % 2 == 1:
                            nc.scalar.activation(op_box[0][:, ct, 0:w],
                                                 op[:, 0:w], CPY)
                        else:
                            nc.vector.tensor_copy(op_box[0][:, ct, 0:w],
                                                  op[:, 0:w])
                    return f
                for ct in range(NCT):
                    parts.append(oppart(ct))

                def store_half(lo, hi):
                    def f():
                        if hi == NCT and not keep_po:
                            del pos[key]
                        # tail stores ride the lighter SP/HWDGE queue (625ns
                        # gen vs ~1.1us SWDGE on Pool) in quarter chunks so
                        # the end-gating transfer is small and starts early
                        eng = nc.sync if tail else nc.gpsimd
                        eng.dma_start(
                            outT_r[:, lo:hi,
                                   rb * RBLK + off:rb * RBLK + off + w],
                            op_box[0][:, lo:hi, 0:w])
                    return f
                if tail:
                    q4 = NCT // 4
                    for j in range(3, -1, -1):
                        parts.insert(3 + (j + 1) * q4,
                                     store_half(j * q4, (j + 1) * q4))
                else:
                    parts.append(store_half(0, NCT))
                return pre, parts

            # Pipeline: cycle i runs attention(i) on ACT/PE while interleaved
            # prep does q-proj(i+1/i+2) and batch-1 K/V proj; the lazy queue
            # carries normalize+out-proj of finished blocks into later
            # blocks' slack.
            for rep in range(reps):
                if rep == 0:
                    pe_warmup()
                for f in stage_q_part(0, halves=qh0 if rep == 0 else 1):
                    f()
                if rep == 0:
                    # Wk + the transpose identity land while the q0
                    # projection runs
                    nc.sync.dma_start(wblob[:, W_WK:W_ID + P],
                                      blob_w[:, W_WK:W_ID + P])
                a0 = stage_a_parts(0)
                a0[0]()          # k-projection of block 0 right away
                if rep == 0:
                    const_late()
                # Block-0 phase split: its sc/exp chains interleave between
                # the pre-loop K/V projections (each group needs only the
                # K-blocks already emitted), so ACT's 16 exps run inside the
                # DMA-paced startup window instead of serializing after it.
                b0_ats = []

                def b0_sc_group(k0, k1):
                    qtb = qtbs[0]
                    for ki in range(k0, k1):
                        scp = ps_sc.tile([P, 2, RBLK], F32, tag="sc", name="sc")
                        for h in (0, 1):
                            hs = slice(h * DK, (h + 1) * DK)
                            nc.tensor.matmul(
                                scp[:, h],
                                KT_sb[hs, ki * P:(ki + 1) * P],
                                qtb[hs, :], start=True, stop=True)
                        at = atp.tile([P, 2, RBLK], DTM, tag="at", name="at")
                        nc.scalar.activation(at[:], scp[:], EXP, scale=0.125)
                        b0_ats.append(at)

                # K-projections of blocks 1-3 go ahead of their V loads: the
                # sc groups (which pace ACT) unblock on K alone, while V is
                # only consumed progressively by b0's attnV later.
                a1 = stage_a_parts(1)
                a2 = stage_a_parts(2)
                a3 = stage_a_parts(3)
                b0_sc_group(0, 4)
                a1[0]()
                # v0 load rides behind k1: sc groups unblock on K alone and
                # b0's attnV only consumes vaug much later
                for f in a0[1:]:
                    f()
                b0_sc_group(4, 8)
                a2[0]()
                for f in a1[1:]:
                    f()
                b0_sc_group(8, 12)
                a3[0]()
                if rep == 0:
                    const_wo()
                for f in a2[1:]:
                    f()
                b0_sc_group(12, NKT)
                for f in a3[1:]:
                    f()
                for f in stage_q_part(1):
                    f()

                def stage_b0_av(prep, pump):
                    po = [ps_out.tile([DK + 1, RBLK], F32, tag=f"po{h}",
                                      name=f"po{h}") for h in (0, 1)]
                    pos[0] = po
                    pi = 0
                    for ki in range(NKT):
                        for h in (0, 1):
                            nc.tensor.matmul(po[h][:], vaugb[0][:, ki, h, :],
                                             b0_ats[ki][:, h],
                                             start=(ki == 0),
                                             stop=(ki == NKT - 1))
                        for _ in range(2):
                            if pi < len(prep):
                                prep[pi]()
                                pi += 1
                            elif work_q:
                                work_q.popleft()()
                            if ki >= NKT // 2:
                                # start block-1 sc/exp once b0's own exps are
                                # mostly drained (shares the ps_sc rotation)
                                pump(0 + look, max_n=1)
                    for p in prep[pi:]:
                        p()
                # Batch-1 K/V prep: one block per cycle starting at rb1 —
                # rb0's PE FIFO must stay clear of matmuls whose input DMA
                # queues behind the batch-0 loads.
                amap = {rb: [RBPB + rb - 1] for rb in range(1, RBPB + 2)}
                def queue_n(lazy_parts, prep):
                    if lazy:
                        work_q.extend(lazy_parts)
                        return prep
                    return lazy_parts + prep

                # ---- flat segment pipeline over blocks 1..NRB-1 ----
                # One global sc/exp cursor runs `look` attnV-steps ahead
                # ACROSS segment (block) boundaries, so the final block's
                # exps start during the previous block's attnV phase and ACT
                # never becomes the end-of-kernel pacer. Emission is gated on
                # the target block's qtb existing (its projection closure ran).
                segments = []
                for rb in range(1, NRB):
                    if rb == NRB - 1 and split_last:
                        off0 = 0
                        for j, sw in enumerate(split_last):
                            segments.append((rb, off0, sw, (rb, j)))
                            off0 += sw
                        assert off0 == RBLK
                    else:
                        segments.append((rb, 0, RBLK, rb))
                flat = [(si, ki) for si in range(len(segments))
                        for ki in range(NKT)]
                pend = deque()
                cursor = [0]

                def pump(upto, max_n=1000):
                    n = 0
                    while cursor[0] < min(upto, len(flat)) and n < max_n:
                        fsi, fki = flat[cursor[0]]
                        frb, foff, fw, _ = segments[fsi]
                        if frb not in qtbs:
                            break
                        pend.append(emit_scexp(frb, fki, foff, fw))
                        cursor[0] += 1
                        n += 1

                # block 0: custom DMA-interleaved path
                prep0 = []
                for a_rb in amap.get(0, []):
                    if RBPB <= a_rb < NRB:
                        prep0 += stage_a_parts(a_rb)
                stage_b0_av(prep0, pump)

                prev_key = (0, 0, RBLK)   # (po_key, off, w) of block 0
                step = 0
                for si, (rb, off, w, po_key) in enumerate(segments):
                    last_seg = si == len(segments) - 1
                    prep = []
                    pk, poff, pw = prev_key
                    prep_pre, lz = stage_n_parts(pk, pk if isinstance(pk, int)
                                                 else pk[0], off=poff, w=pw)
                    prep = queue_n(lz, prep)
                    if off == 0:
                        for a_rb in amap.get(rb, []):
                            if RBPB <= a_rb < NRB:
                                prep += stage_a_parts(a_rb)
                        if rb + 1 < NRB:
                            prep += stage_q_part(rb + 1)
                    # hold back RSV lazy closures until the final two
                    # segments: the endgame attnV is ACT-paced and otherwise
                    # has no PE filler left
                    rsv = RSV if si < len(segments) - 2 else 0
                    pi = 0
                    b = rb // RBPB
                    for p in prep_pre:
                        p()
                    po = [ps_out.tile([DK + 1, RBLK], F32, tag=f"po{h}",
                                      name=f"po{h}") for h in (0, 1)]
                    pos[po_key] = po
                    for ki in range(NKT):
                        pump(step + 1)      # guarantee this step's at-tile
                        cur = pend.popleft()
                        step += 1
                        for h in (0, 1):
                            nc.tensor.matmul(po[h][:, 0:w],
                                             vaugb[b][:, ki, h, :],
                                             cur[:, h, 0:w],
                                             start=(ki == 0),
                                             stop=(ki == NKT - 1))
                        # interleave up to 2 closures per ki step with at most
                        # one lookahead emission after each, so deep lookahead
                        # never puts back-to-back exp-paced sc groups in the
                        # PE FIFO without filler between them
                        for _ in range(2):
                            if pi < len(prep):
                                prep[pi]()
                                pi += 1
                            elif len(work_q) > rsv:
                                work_q.popleft()()
                            pump(step + look, max_n=1)
                    for p in prep[pi:]:
                        p()
                    prev_key = (po_key, off, w)
                # drain: final segment's normalize + all remaining lazy work
                pk, poff, pw = prev_key
                pre, lz = stage_n_parts(pk, NRB - 1, off=poff, w=pw,
                                        tail=True)
                for f in pre:
                    f()
                while work_q:
                    work_q.popleft()()
                for f in lz:
                    f()

    nc.compile()
    return nc


def host_prepare(q, k, v, Wq, bq, Wk, bk, Wv, bv, Wo, bo, B=2, S=2048,
                 in_dt="f16"):
    """Build per-core in_maps. Returns (in_maps, postprocess)."""
    R = B * S
    f32 = np.float32
    f16 = np.float16
    fin = f16 if in_dt == "f16" else f32
    qT = np.ascontiguousarray(q.reshape(R, D).T).astype(fin, copy=False)
    kT = np.ascontiguousarray(k.reshape(R, D).T).astype(fin, copy=False)
    vT = np.ascontiguousarray(v.reshape(R, D).T).astype(fin, copy=False)
    Wq, Wk, Wv, Wo = (np.asarray(x, f32) for x in (Wq, Wk, Wv, Wo))
    bqa, bka, bva, boa = (np.asarray(x, f32) for x in (bq, bk, bv, bo))

    def pack_w(w_slice):
        # W[dd] is [128 outdims, D]; kernel wants tile [p, do, d] with
        # tile[p, do, d] = W[dd].T[do*128+p, d]; rows packed p-major so each
        # DMA descriptor is one contiguous 2KB row segment.
        wT = w_slice.T.reshape(DO, P, P)          # [do, p, d]
        return wT.transpose(1, 0, 2).reshape(P, DO * P)

    in_maps = []
    for c in range(NCORES):
        dd = slice(P * c, P * (c + 1))
        blob_w = np.zeros((P, W_COLS), f32)
        blob_w[:, W_WQ:W_WQ + DO * P] = pack_w(Wq[dd])
        blob_w[:, W_WK:W_WK + DO * P] = pack_w(Wk[dd])
        blob_w[:, W_ID:W_ID + P] = np.eye(P, dtype=f32)
        blob_m = np.zeros((DK + 1, M_COLS), f32)
        blob_m[0, M_BQ:M_BQ + P] = bqa[dd]
        blob_m[0, M_BK:M_BK + P] = bka[dd]
        blob_m[0, M_ONES:M_ONES + RBLK] = 1.0
        blob_m[:, M_ONESZ:M_ONESZ + DK] = 1.0
        blob_l = np.zeros((P, L_COLS), f32)
        blob_l[:, L_WV:L_WV + DO * P] = pack_w(Wv[dd])
        # wo tile is [p(concat dim), ct, c] = Wo[:, dd].T[p, ct*128+c]
        blob_l[:, L_WO:L_WO + DO * P] = Wo[:, dd].T
        blob_l[0, L_BV:L_BV + P] = bva[dd]
        in_maps.append({
            "qT": qT, "kT": kT, "vT": vT,
            "blob_w": blob_w.astype(fin, copy=False),
            "blob_m": blob_m.astype(fin, copy=False),
            "blob_l": blob_l.astype(f16, copy=False),
        })

    def postprocess(results):
        acc = np.zeros((D, R), np.float32)
        for c in range(NCORES):
            acc += results[c]["outT"].astype(np.float32)
        out = acc.T + boa
        return out.astype(f32).reshape(B, S, D)

    return in_maps, postprocess


# ---------------------------------------------------------------------------
# Harness entry point: full inputs in, full output out.
# ---------------------------------------------------------------------------
_BUILD_CACHE = {}


def kernel(q, k, v, Wq, bq, Wk, bk, Wv, bv, Wo, bo, mask=0, **_unused):
    from concourse import bass_utils

    has_bias = any(np.any(np.asarray(b_)) for b_ in (bq, bk, bv))
    nc = _BUILD_CACHE.get(("nc", has_bias))
    if nc is None:
        nc = build_kernel(B=2, S=2048, with_bias=has_bias)
        _BUILD_CACHE[("nc", has_bias)] = nc

    args = [np.asarray(x, np.float32) for x in
            (q, k, v, Wq, bq, Wk, bk, Wv, bv, Wo, bo)]
    in_maps, post = host_prepare(*args)
    res = bass_utils.run_bass_kernel_spmd(nc, in_maps, core_ids=list(range(8)))
    return post(res.results)

